# revision 33
# baseline (speedup 1.0000x reference)
"""Causal single-head attention on 8 Trainium2 NeuronCores (Bass/Tile).

Problem: x [4, 2048, 1024], W_{q,k,v} [1024, 1024] (torch Linear layout,
y = x @ W.T), causal softmax(QK^T/sqrt(D)) @ V  ->  [4, 2048, 1024] fp32.

Sharding (uniform SPMD program, per-core data only):
  core c -> batch b = c//2, key-parity h = c%2.
  Each core computes attention for ALL 2048 queries of its batch against
  the 1024 keys with original index = h (mod 2) ("virtual" keys k' with
  global key = 2k' + h), flash-style transposed (S^T[k', q] tiles),
  unnormalized: O_part = sum_k exp(s) V, l_part = sum_k exp(s). Host
  combines: out[b] = (O_0 + O_1) / (l_0 + l_1). Causality over virtual
  keys makes every (k'-tile j, q-chunk i) block with j < i fully allowed
  and the j == i block maskable with one slot-independent pattern
  (allowed iff q_l >= 2*k_l + h), so all 8 core programs are IDENTICAL.

  Wq/Wk folded on host: scores = x_q @ (Wq^T Wk) @ x_k^T, so the device
  does 2 projections (C = x_k G, V = x_k Wv^T), not 3.

Precision: everything bf16 on the PE (1 cyc/row, same as fp32r, half
the DMA/SBUF/ldweights cost), fp32 PSUM accumulation. fp8 DoubleRow
was evaluated and rejected: on this data the score distribution is
heavy-tailed (max s/32 = 7.3) and host emulation put fp8 attention at
1.9-3.4e-2 max-rel error vs the 2e-2 gate. Attention S^T blocks are
computed for SLOT PAIRS ([128 k', 512 q] covering two 256-query slots)
to halve S instruction and exp counts. exp carries bias=-1.5 (cancels
in O/l; kept for numerical headroom).

Schedule: V projection first (first matmul needs only ~1.25 MB of DMA,
issued as few fat descriptor sets - issue costs ~700ns each), then C,
with the query-side x^T streamed behind. Attention is one global
software pipeline: S units run LOOK units ahead of the AV consumers
across slot boundaries; scalar does only exp, vector only drains,
gpsimd applies the 0/1 causal mask post-exp (it cannot touch PSUM).
The last slot accumulates O in dv-halves so the first half's
drain+DMA overlaps the second half's matmuls.
"""

import numpy as np
import ml_dtypes

import concourse.mybir as mybir
import concourse.tile as tile
from concourse import bacc
from concourse.bass_utils import run_bass_kernel_spmd

F32 = mybir.dt.float32
BF = mybir.dt.bfloat16
BF_NP = ml_dtypes.bfloat16

B, S, D = 4, 2048, 1024
NP = 128  # partitions
DP = D // NP  # 8 contraction-dim tiles
ET = D // NP  # 8 output-dim tiles
KP = S // 2  # 1024 keys per core
KT = KP // NP  # 8 key tiles
QCH = 256  # per-slot query width
QW = 2 * QCH  # paired-slot width
NSLOT = S // QCH  # 8 slots
NPAIR = NSLOT // 2  # 4 slot pairs
SCALE = 1.0 / 32.0  # 1/sqrt(D)
EBIAS = -1.5  # exp bias: keeps fp8 weights < 240 (cancels in O/l)
LOOK = 4  # S-unit runahead, limited by 5 PSUM score banks

_NC_CACHE = {}


def _build_nc():
    nc = bacc.Bacc(None, target_bir_lowering=False)

    # host-pretiled inputs, contiguous per partition for fat few-issue DMAs
    xt = nc.dram_tensor("xt", [NP, NPAIR, DP, QW], BF, kind="ExternalInput")
    xka = nc.dram_tensor("xka", [4, NP, DP, NP], BF, kind="ExternalInput")
    xkb = nc.dram_tensor("xkb", [NP, DP, 512], BF, kind="ExternalInput")
    # wgt = Wk^T @ Wq (host-folded QK^T kernel matrix): [p, dp, e]
    wgt = nc.dram_tensor("wgt", [NP, DP, D], BF, kind="ExternalInput")
    # wvt = Wv^T split in 256-col quarters: [q4, p, dp, e']
    wvt = nc.dram_tensor("wvt", [4, NP, DP, 256], BF, kind="ExternalInput")
    # 0/1 causal mask: left half = diag pattern, right half = ones
    mask = nc.dram_tensor("mask", [NP, QW], BF, kind="ExternalInput")
    ones = nc.dram_tensor("ones", [NP, 2], BF, kind="ExternalInput")
    o_out = nc.dram_tensor("o", [S, D], F32, kind="ExternalOutput")
    l_out = nc.dram_tensor("l", [NSLOT, 2, QCH], F32, kind="ExternalOutput")

    o_r = o_out.rearrange("(t p) d -> p t d", p=NP)  # [128, 16, 1024]

    with tile.TileContext(nc) as tc:
        with tc.tile_pool(name="res", bufs=1) as res:
            xt_res = res.tile([NP, NPAIR, DP, QW], BF)  # 16KB/p
            ct_res = res.tile([NP, ET, KP], BF)  # 16KB/p
            v_res = res.tile([NP, KT, D], BF)  # 16KB/p
            t_mask = res.tile([NP, QW], BF)
            t_ones = res.tile([NP, 2], BF)
            t_bias = res.tile([NP, 1], F32)
            nc.gpsimd.memset(t_bias[:], EBIAS)

            # ---------------- projections ----------------
            with (
                tc.tile_pool(name="wp", bufs=1) as wp,
                tc.tile_pool(name="xp", bufs=1) as xp,
                tc.tile_pool(name="pps", bufs=4, space="PSUM") as pps,
            ):
                wv_sb = wp.tile([NP, 4, DP, 256], BF, tag="wv", name="wv")
                wg_sb = wp.tile([NP, DP, D], BF, tag="wg", name="wg")
                xk_sb = [
                    xp.tile([NP, DP, 512], BF, tag=f"xk{s_}", name=f"xk{s_}")
                    for s_ in range(2)
                ]

                # DMA issue order = urgency order. All inputs stay on ONE
                # queue: a second queue steals bandwidth from the critical
                # first-chain pieces (measured: split queues pushed the
                # first chain's data from ~11us to ~18us).
                nc.sync.dma_start(wv_sb[:, 0], wvt[0])
                nc.sync.dma_start(xk_sb[0][:, :, 0:NP], xka[0])
                nc.sync.dma_start(wv_sb[:, 1], wvt[1])
                for sub in range(1, 4):
                    nc.sync.dma_start(
                        xk_sb[0][:, :, sub * NP : (sub + 1) * NP], xka[sub]
                    )
                nc.sync.dma_start(wv_sb[:, 2], wvt[2])
                nc.sync.dma_start(wv_sb[:, 3], wvt[3])
                nc.sync.dma_start(wg_sb[:], wgt[:])
                nc.sync.dma_start(xk_sb[1][:], xkb[:])
                nc.sync.dma_start(t_mask[:], mask[:])
                nc.sync.dma_start(t_ones[:], ones[:])
                nc.sync.dma_start(xt_res[:], xt[:])

                # PE p-state warm-up: the tensor engine runs ~2x slow for
                # its first ~3us of activity. Burn that ramp on dummy
                # matmuls over scratch SBUF during the initial DMA wait so
                # the first real chains run at full clock.
                warm = wp.tile([NP, 512], BF, tag="warm", name="warm")
                nc.gpsimd.memset(warm[:], 0.25)
                wps = pps.tile([NP, 512], F32, tag="pps", name="warmps")
                for r in range(10):
                    nc.tensor.matmul(
                        wps[:], warm[:, 0:NP], warm[:],
                        start=(r == 0), stop=(r == 9),
                    )

                def v_proj256(kt_i, q4):
                    xc = xk_sb[kt_i // 4]
                    sub = kt_i % 4
                    ps = pps.tile([NP, 256], F32, tag="pps4", name=f"psv{kt_i}_{q4}")
                    for dp in range(DP):
                        nc.tensor.matmul(
                            ps[:],
                            xc[:, dp, sub * NP : (sub + 1) * NP],
                            wv_sb[:, q4, dp, :],
                            start=(dp == 0),
                            stop=(dp == DP - 1),
                        )
                    nc.vector.tensor_copy(
                        v_res[:, kt_i, q4 * 256 : (q4 + 1) * 256], ps[:]
                    )

                def v_proj512(kt_i, dv):
                    xc = xk_sb[kt_i // 4]
                    sub = kt_i % 4
                    ps = pps.tile([NP, 512], F32, tag="pps", name=f"psw{kt_i}_{dv}")
                    for dp in range(DP):
                        nc.tensor.matmul(
                            ps[:],
                            xc[:, dp, sub * NP : (sub + 1) * NP],
                            wv_sb[:, 2 * dv : 2 * dv + 2, dp, :],
                            start=(dp == 0),
                            stop=(dp == DP - 1),
                        )
                    nc.vector.tensor_copy(
                        v_res[:, kt_i, dv * 512 : (dv + 1) * 512], ps[:]
                    )

                def c_proj(ks, et):
                    ps = pps.tile([NP, 512], F32, tag="pps", name=f"psk{ks}_{et}")
                    for dp in range(DP):
                        nc.tensor.matmul(
                            ps[:],
                            wg_sb[:, dp, et * NP : (et + 1) * NP],
                            xk_sb[ks][:, dp, :],
                            start=(dp == 0),
                            stop=(dp == DP - 1),
                        )
                    nc.vector.tensor_copy(
                        ct_res[:, et, ks * 512 : (ks + 1) * 512], ps[:]
                    )

                v_proj256(0, 0)
                v_proj256(0, 1)
                for kt_i in range(1, 4):
                    v_proj512(kt_i, 0)
                for kt_i in range(4):
                    v_proj512(kt_i, 1)
                for et in range(ET):
                    c_proj(0, et)
                for kt_i in range(4, 8):
                    v_proj512(kt_i, 0)
                    v_proj512(kt_i, 1)
                for et in range(ET):
                    c_proj(1, et)

            # ---------------- attention ----------------
            # S production units per slot-pair pi (slots 2pi, 2pi+1):
            #   j <= 2pi     : paired-slot [128, QW] (j == 2pi is diag for
            #                  slot 2pi via mask; fully allowed for 2pi+1)
            #   j == 2pi + 1 : single-slot [128, QCH] (diag for slot 2pi+1)
            sunits = [(pi, j) for pi in range(NPAIR) for j in range(2 * pi + 2)]
            soff = [0, 2, 6, 12]  # global index of (pi, 0)
            with (
                tc.tile_pool(name="pbp", bufs=8) as pbp,
                tc.tile_pool(name="pop", bufs=2) as pop,
                tc.tile_pool(name="prp", bufs=2) as prp,
                tc.tile_pool(name="ost", bufs=2) as ost,
                tc.tile_pool(name="sps", bufs=5, space="PSUM") as sps,
                tc.tile_pool(name="ops", bufs=1, space="PSUM") as ops,
                tc.tile_pool(name="lps", bufs=1, space="PSUM") as lps,
            ):
                pb_t = {}
                pbo_t = {}
                o_ps = {}
                l_ps = {}

                def s_unit(k):
                    pi, j = sunits[k]
                    s_ps = sps.tile([NP, QW], F32, tag="s", name=f"s{pi}_{j}")
                    if j == 2 * pi + 1:  # odd diag: single slot
                        for et in range(ET):
                            nc.tensor.matmul(
                                s_ps[:, 0:QCH],
                                ct_res[:, et, j * NP : (j + 1) * NP],
                                xt_res[:, pi, et, QCH:QW],
                                start=(et == 0),
                                stop=(et == ET - 1),
                            )
                        praw = prp.tile([NP, QCH], BF, tag="pro", name=f"pro{pi}")
                        nc.scalar.activation(
                            out=praw[:],
                            in_=s_ps[:, 0:QCH],
                            func=mybir.ActivationFunctionType.Exp,
                            scale=SCALE,
                            bias=t_bias[:],
                        )
                        p_t = pop.tile([NP, QCH], BF, tag="pbo", name=f"pbo{pi}")
                        nc.gpsimd.tensor_mul(p_t[:], praw[:], t_mask[:, 0:QCH])
                        pbo_t[pi] = p_t
                    else:  # paired slot [128, QW]
                        for et in range(ET):
                            nc.tensor.matmul(
                                s_ps[:],
                                ct_res[:, et, j * NP : (j + 1) * NP],
                                xt_res[:, pi, et, :],
                                start=(et == 0),
                                stop=(et == ET - 1),
                            )
                        p_t = pbp.tile([NP, QW], BF, tag="pb", name=f"pb{pi}_{j}")
                        if j == 2 * pi:  # diag for slot 2pi: mask left half
                            praw = prp.tile([NP, QW], BF, tag="pre", name=f"pre{pi}")
                            nc.scalar.activation(
                                out=praw[:],
                                in_=s_ps[:],
                                func=mybir.ActivationFunctionType.Exp,
                                scale=SCALE,
                                bias=t_bias[:],
                            )
                            nc.gpsimd.tensor_mul(p_t[:], praw[:], t_mask[:])
                        else:
                            nc.scalar.activation(
                                out=p_t[:],
                                in_=s_ps[:],
                                func=mybir.ActivationFunctionType.Exp,
                                scale=SCALE,
                                bias=t_bias[:],
                            )
                        pb_t[(pi, j)] = p_t

                sp = 0

                def ensure(need_idx):
                    nonlocal sp
                    target = min(need_idx + 1 + LOOK, len(sunits))
                    while sp < target:
                        s_unit(sp)
                        sp += 1

                def drain_half(sl, dv, o_cur, do_l):
                    if do_l:
                        lt = ost.tile([2, QCH], F32, tag="lt", name=f"lt{sl}")
                        nc.vector.tensor_copy(lt[:], l_ps[sl][:])
                        nc.sync.dma_start(l_out[sl], lt[:])
                    for q in range(2):
                        ot = ost.tile(
                            [NP, 512], F32, tag=f"ot{q}", name=f"ot{sl}_{dv}_{q}"
                        )
                        tailq = sl == NSLOT - 1 and q == 1
                        if tailq:
                            nc.scalar.activation(
                                out=ot[:], in_=o_cur[q][:],
                                func=mybir.ActivationFunctionType.Copy,
                            )
                            nc.scalar.dma_start(
                                o_r[:, sl * 2 + q, dv * 512 : (dv + 1) * 512],
                                ot[:],
                            )
                        else:
                            nc.vector.tensor_copy(ot[:], o_cur[q][:])
                            nc.sync.dma_start(
                                o_r[:, sl * 2 + q, dv * 512 : (dv + 1) * 512],
                                ot[:],
                            )

                # Every slot accumulates O in two dv-half passes over its j
                # blocks: halves O's PSUM footprint (2 banks, enabling the
                # 5-deep score-bank runahead) and overlaps each dv0 drain
                # with the dv1 matmuls. Pass dv1 prefetches the NEXT slot's
                # S units between its AV blocks.
                NS = len(sunits)
                for sl in range(NSLOT):
                    pi, inp = sl // 2, sl % 2
                    qo = inp * QCH
                    soff_next = soff[(sl + 1) // 2] if sl + 1 < NSLOT else NS - 1
                    l_ps[sl] = lps.tile([2, QCH], F32, tag="l", name=f"l{sl}")
                    for dv in range(2):
                        o_cur = [
                            ops.tile(
                                [NP, 512], F32, tag=f"o{q}", name=f"o{sl}_{dv}_{q}"
                            )
                            for q in range(2)
                        ]
                        for j in range(sl + 1):
                            if dv == 0:
                                ensure(soff[pi] + j)
                            else:
                                ensure(min(soff_next + j, NS - 1))
                            first, last = (j == 0), (j == sl)
                            if inp == 1 and j == sl:
                                pt, coff = pbo_t[pi], 0
                            else:
                                pt, coff = pb_t[(pi, j)], qo
                            if dv == 0:
                                nc.tensor.matmul(
                                    l_ps[sl][:],
                                    t_ones[:],
                                    pt[:, coff : coff + QCH],
                                    start=first,
                                    stop=last,
                                )
                            for q in range(2):
                                nc.tensor.matmul(
                                    o_cur[q][:],
                                    pt[:, coff + q * NP : coff + (q + 1) * NP],
                                    v_res[:, j, dv * 512 : (dv + 1) * 512],
                                    start=first,
                                    stop=last,
                                )
                        drain_half(sl, dv, o_cur, do_l=(dv == 0))
    nc.compile()
    return nc


def _get_nc():
    if "nc" not in _NC_CACHE:
        _NC_CACHE["nc"] = _build_nc()
    return _NC_CACHE["nc"]


def kernel(x, W_query, W_key, W_value):
    x = np.asarray(x, dtype=np.float32)
    # fold Wq/Wk: scores = x_q @ (Wq^T Wk) @ x_k^T; device computes
    # C^T[e, k'] with stationary wgt[d, e] = (Wk^T @ Wq)[d, e]
    G = (
        np.asarray(W_key, dtype=np.float64).T @ np.asarray(W_query, dtype=np.float64)
    ).astype(BF_NP)
    wgt_a = np.ascontiguousarray(G.reshape(DP, NP, D).transpose(1, 0, 2))
    wvt_f = np.asarray(W_value, dtype=np.float32).T.astype(BF_NP)  # [D, D]
    wvt_a = np.ascontiguousarray(
        wvt_f.reshape(DP, NP, 4, 256).transpose(2, 1, 0, 3)
    )

    ones_a = np.ones((NP, 2), dtype=BF_NP)
    k_l = np.arange(NP)[:, None]
    q_l = np.arange(QCH)[None, :]

    in_maps = []
    for c in range(8):
        b, h = c // 2, c % 2
        xb = x[b]
        # queries bf16: xt[p, pi, dp, qw] = x[b, pi*512+qw, dp*128+p]
        xt_t = np.ascontiguousarray(
            xb.reshape(NPAIR, QW, DP, NP).transpose(3, 0, 2, 1).astype(BF_NP)
        )
        # keys (parity h): fine slabs for keys 0-511, coarse for 512-1023
        xkv = xb[h::2].astype(BF_NP)  # [KP, D]
        xka_t = np.ascontiguousarray(
            xkv[:512].reshape(4, NP, DP, NP).transpose(0, 3, 2, 1)
        )
        xkb_t = np.ascontiguousarray(
            xkv[512:].reshape(512, DP, NP).transpose(2, 1, 0)
        )
        mask_a = np.ones((NP, QW), dtype=BF_NP)
        mask_a[:, 0:QCH] = (q_l >= 2 * k_l + h).astype(BF_NP)
        in_maps.append(
            {
                "xt": xt_t,
                "xka": xka_t,
                "xkb": xkb_t,
                "wgt": wgt_a,
                "wvt": wvt_a,
                "mask": mask_a,
                "ones": ones_a,
            }
        )

    nc = _get_nc()
    res = run_bass_kernel_spmd(nc, in_maps, core_ids=list(range(8)))
    _NC_CACHE["last_res"] = res
    if res.exec_time_ns is not None:
        print(f"HW exec time: {res.exec_time_ns} ns")

    out = np.empty((B, S, D), dtype=np.float32)
    for b in range(B):
        o0 = res.results[2 * b]["o"]
        o1 = res.results[2 * b + 1]["o"]
        l0 = res.results[2 * b]["l"][:, 0, :].reshape(S, 1)
        l1 = res.results[2 * b + 1]["l"][:, 0, :].reshape(S, 1)
        out[b] = (o0 + o1) / (l0 + l1)
    return out


# revision 34
# speedup vs baseline: 1.0199x; 1.0199x over previous
"""Causal single-head attention on 8 Trainium2 NeuronCores (Bass/Tile).

Problem: x [4, 2048, 1024], W_{q,k,v} [1024, 1024] (torch Linear layout,
y = x @ W.T), causal softmax(QK^T/sqrt(D)) @ V  ->  [4, 2048, 1024] fp32.

Sharding (uniform SPMD program, per-core data only):
  core c -> batch b = c//2, key-parity h = c%2.
  Each core computes attention for ALL 2048 queries of its batch against
  the 1024 keys with original index = h (mod 2) ("virtual" keys k' with
  global key = 2k' + h), flash-style transposed (S^T[k', q] tiles),
  unnormalized: O_part = sum_k exp(s) V, l_part = sum_k exp(s). Host
  combines: out[b] = (O_0 + O_1) / (l_0 + l_1). Causality over virtual
  keys makes every (k'-tile j, q-chunk i) block with j < i fully allowed
  and the j == i block maskable with one slot-independent pattern
  (allowed iff q_l >= 2*k_l + h), so all 8 core programs are IDENTICAL.

  Wq/Wk folded on host: scores = x_q @ (Wq^T Wk) @ x_k^T, so the device
  does 2 projections (C = x_k G, V = x_k Wv^T), not 3.

Precision: everything bf16 on the PE (1 cyc/row, same as fp32r, half
the DMA/SBUF/ldweights cost), fp32 PSUM accumulation. fp8 DoubleRow
was evaluated and rejected: on this data the score distribution is
heavy-tailed (max s/32 = 7.3) and host emulation put fp8 attention at
1.9-3.4e-2 max-rel error vs the 2e-2 gate. Attention S^T blocks are
computed for SLOT PAIRS ([128 k', 512 q] covering two 256-query slots)
to halve S instruction and exp counts. exp carries bias=-1.5 (cancels
in O/l; kept for numerical headroom).

Schedule: V projection first (first matmul needs only ~1.25 MB of DMA,
issued as few fat descriptor sets - issue costs ~700ns each), then C,
with the query-side x^T streamed behind. Attention is one global
software pipeline: S units run LOOK units ahead of the AV consumers
across slot boundaries; scalar does only exp, vector only drains,
gpsimd applies the 0/1 causal mask post-exp (it cannot touch PSUM).
The last slot accumulates O in dv-halves so the first half's
drain+DMA overlaps the second half's matmuls.
"""

import numpy as np
import ml_dtypes

import concourse.mybir as mybir
import concourse.tile as tile
from concourse import bacc
from concourse.bass_utils import run_bass_kernel_spmd

F32 = mybir.dt.float32
BF = mybir.dt.bfloat16
BF_NP = ml_dtypes.bfloat16

B, S, D = 4, 2048, 1024
NP = 128  # partitions
DP = D // NP  # 8 contraction-dim tiles
ET = D // NP  # 8 output-dim tiles
KP = S // 2  # 1024 keys per core
KT = KP // NP  # 8 key tiles
QCH = 256  # per-slot query width
QW = 2 * QCH  # paired-slot width
NSLOT = S // QCH  # 8 slots
NPAIR = NSLOT // 2  # 4 slot pairs
SCALE = 1.0 / 32.0  # 1/sqrt(D)
EBIAS = -1.5  # exp bias: keeps fp8 weights < 240 (cancels in O/l)
LOOK = 4  # S-unit runahead, limited by 5 PSUM score banks

_NC_CACHE = {}


def _build_nc():
    nc = bacc.Bacc(None, target_bir_lowering=False)

    # host-pretiled inputs, contiguous per partition for fat few-issue DMAs
    xt = nc.dram_tensor("xt", [NP, NPAIR, DP, QW], BF, kind="ExternalInput")
    xka = nc.dram_tensor("xka", [4, NP, DP, NP], BF, kind="ExternalInput")
    xkb = nc.dram_tensor("xkb", [NP, DP, 512], BF, kind="ExternalInput")
    # wgt = Wk^T @ Wq (host-folded QK^T kernel matrix): [p, dp, e]
    wgt = nc.dram_tensor("wgt", [NP, DP, D], BF, kind="ExternalInput")
    # wvt = Wv^T split in 256-col quarters: [q4, p, dp, e']
    wvt = nc.dram_tensor("wvt", [4, NP, DP, 256], BF, kind="ExternalInput")
    # 0/1 causal mask: left half = diag pattern, right half = ones
    mask = nc.dram_tensor("mask", [NP, QW], BF, kind="ExternalInput")
    ones = nc.dram_tensor("ones", [NP, 2], BF, kind="ExternalInput")
    o_out = nc.dram_tensor("o", [S, D], F32, kind="ExternalOutput")
    l_out = nc.dram_tensor("l", [NSLOT, 2, QCH], F32, kind="ExternalOutput")

    o_r = o_out.rearrange("(t p) d -> p t d", p=NP)  # [128, 16, 1024]

    with tile.TileContext(nc) as tc:
        with tc.tile_pool(name="res", bufs=1) as res:
            xt_res = res.tile([NP, NPAIR, DP, QW], BF)  # 16KB/p
            ct_res = res.tile([NP, ET, KP], BF)  # 16KB/p
            v_res = res.tile([NP, KT, D], BF)  # 16KB/p
            t_mask = res.tile([NP, QW], BF)
            t_ones = res.tile([NP, 2], BF)
            t_bias = res.tile([NP, 1], F32)
            nc.gpsimd.memset(t_bias[:], EBIAS)

            # ---------------- projections ----------------
            with (
                tc.tile_pool(name="wp", bufs=1) as wp,
                tc.tile_pool(name="xp", bufs=1) as xp,
                tc.tile_pool(name="pps", bufs=4, space="PSUM") as pps,
            ):
                wv_sb = wp.tile([NP, 4, DP, 256], BF, tag="wv", name="wv")
                wg_sb = wp.tile([NP, DP, D], BF, tag="wg", name="wg")
                xk_sb = [
                    xp.tile([NP, DP, 512], BF, tag=f"xk{s_}", name=f"xk{s_}")
                    for s_ in range(2)
                ]

                # DMA issue order = urgency order. All inputs stay on ONE
                # queue: a second queue steals bandwidth from the critical
                # first-chain pieces (measured: split queues pushed the
                # first chain's data from ~11us to ~18us).
                nc.sync.dma_start(wv_sb[:, 0], wvt[0])
                nc.sync.dma_start(xk_sb[0][:, :, 0:NP], xka[0])
                nc.sync.dma_start(wv_sb[:, 1], wvt[1])
                for sub in range(1, 4):
                    nc.sync.dma_start(
                        xk_sb[0][:, :, sub * NP : (sub + 1) * NP], xka[sub]
                    )
                nc.sync.dma_start(wv_sb[:, 2], wvt[2])
                nc.sync.dma_start(wv_sb[:, 3], wvt[3])
                nc.sync.dma_start(wg_sb[:], wgt[:])
                nc.sync.dma_start(xk_sb[1][:], xkb[:])
                nc.sync.dma_start(t_mask[:], mask[:])
                nc.sync.dma_start(t_ones[:], ones[:])
                nc.sync.dma_start(xt_res[:], xt[:])

                # PE p-state warm-up: the tensor engine runs ~2x slow for
                # its first ~3us of activity. Burn that ramp on dummy
                # matmuls over scratch SBUF during the initial DMA wait so
                # the first real chains run at full clock.
                warm = wp.tile([NP, 512], BF, tag="warm", name="warm")
                nc.gpsimd.memset(warm[:], 0.25)
                wps = pps.tile([NP, 512], F32, tag="pps", name="warmps")
                for r in range(10):
                    nc.tensor.matmul(
                        wps[:], warm[:, 0:NP], warm[:],
                        start=(r == 0), stop=(r == 9),
                    )

                def v_proj256(kt_i, q4):
                    xc = xk_sb[kt_i // 4]
                    sub = kt_i % 4
                    ps = pps.tile([NP, 256], F32, tag="pps4", name=f"psv{kt_i}_{q4}")
                    for dp in range(DP):
                        nc.tensor.matmul(
                            ps[:],
                            xc[:, dp, sub * NP : (sub + 1) * NP],
                            wv_sb[:, q4, dp, :],
                            start=(dp == 0),
                            stop=(dp == DP - 1),
                        )
                    nc.vector.tensor_copy(
                        v_res[:, kt_i, q4 * 256 : (q4 + 1) * 256], ps[:]
                    )

                def v_proj512(kt_i, dv):
                    xc = xk_sb[kt_i // 4]
                    sub = kt_i % 4
                    ps = pps.tile([NP, 512], F32, tag="pps", name=f"psw{kt_i}_{dv}")
                    for dp in range(DP):
                        nc.tensor.matmul(
                            ps[:],
                            xc[:, dp, sub * NP : (sub + 1) * NP],
                            wv_sb[:, 2 * dv : 2 * dv + 2, dp, :],
                            start=(dp == 0),
                            stop=(dp == DP - 1),
                        )
                    nc.vector.tensor_copy(
                        v_res[:, kt_i, dv * 512 : (dv + 1) * 512], ps[:]
                    )

                def c_proj(ks, et):
                    ps = pps.tile([NP, 512], F32, tag="pps", name=f"psk{ks}_{et}")
                    for dp in range(DP):
                        nc.tensor.matmul(
                            ps[:],
                            wg_sb[:, dp, et * NP : (et + 1) * NP],
                            xk_sb[ks][:, dp, :],
                            start=(dp == 0),
                            stop=(dp == DP - 1),
                        )
                    nc.vector.tensor_copy(
                        ct_res[:, et, ks * 512 : (ks + 1) * 512], ps[:]
                    )

                v_proj256(0, 0)
                v_proj256(1, 0)  # needs only sub1: absorbs the wv-q1 wait
                v_proj256(0, 1)
                v_proj256(1, 1)
                for kt_i in range(2, 4):
                    v_proj512(kt_i, 0)
                for kt_i in range(4):
                    v_proj512(kt_i, 1)
                for et in range(ET):
                    c_proj(0, et)
                for kt_i in range(4, 8):
                    v_proj512(kt_i, 0)
                    v_proj512(kt_i, 1)
                for et in range(ET):
                    c_proj(1, et)

            # ---------------- attention ----------------
            # S production units per slot-pair pi (slots 2pi, 2pi+1):
            #   j <= 2pi     : paired-slot [128, QW] (j == 2pi is diag for
            #                  slot 2pi via mask; fully allowed for 2pi+1)
            #   j == 2pi + 1 : single-slot [128, QCH] (diag for slot 2pi+1)
            sunits = [(pi, j) for pi in range(NPAIR) for j in range(2 * pi + 2)]
            soff = [0, 2, 6, 12]  # global index of (pi, 0)
            with (
                tc.tile_pool(name="pbp", bufs=8) as pbp,
                tc.tile_pool(name="pop", bufs=2) as pop,
                tc.tile_pool(name="prp", bufs=2) as prp,
                tc.tile_pool(name="ost", bufs=2) as ost,
                tc.tile_pool(name="sps", bufs=5, space="PSUM") as sps,
                tc.tile_pool(name="ops", bufs=1, space="PSUM") as ops,
                tc.tile_pool(name="lps", bufs=1, space="PSUM") as lps,
            ):
                pb_t = {}
                pbo_t = {}
                o_ps = {}
                l_ps = {}

                def s_unit(k):
                    pi, j = sunits[k]
                    s_ps = sps.tile([NP, QW], F32, tag="s", name=f"s{pi}_{j}")
                    if j == 2 * pi + 1:  # odd diag: single slot
                        for et in range(ET):
                            nc.tensor.matmul(
                                s_ps[:, 0:QCH],
                                ct_res[:, et, j * NP : (j + 1) * NP],
                                xt_res[:, pi, et, QCH:QW],
                                start=(et == 0),
                                stop=(et == ET - 1),
                            )
                        praw = prp.tile([NP, QCH], BF, tag="pro", name=f"pro{pi}")
                        nc.scalar.activation(
                            out=praw[:],
                            in_=s_ps[:, 0:QCH],
                            func=mybir.ActivationFunctionType.Exp,
                            scale=SCALE,
                            bias=t_bias[:],
                        )
                        p_t = pop.tile([NP, QCH], BF, tag="pbo", name=f"pbo{pi}")
                        nc.gpsimd.tensor_mul(p_t[:], praw[:], t_mask[:, 0:QCH])
                        pbo_t[pi] = p_t
                    else:  # paired slot [128, QW]
                        for et in range(ET):
                            nc.tensor.matmul(
                                s_ps[:],
                                ct_res[:, et, j * NP : (j + 1) * NP],
                                xt_res[:, pi, et, :],
                                start=(et == 0),
                                stop=(et == ET - 1),
                            )
                        p_t = pbp.tile([NP, QW], BF, tag="pb", name=f"pb{pi}_{j}")
                        if j == 2 * pi:  # diag for slot 2pi: mask left half
                            praw = prp.tile([NP, QW], BF, tag="pre", name=f"pre{pi}")
                            nc.scalar.activation(
                                out=praw[:],
                                in_=s_ps[:],
                                func=mybir.ActivationFunctionType.Exp,
                                scale=SCALE,
                                bias=t_bias[:],
                            )
                            nc.gpsimd.tensor_mul(p_t[:], praw[:], t_mask[:])
                        else:
                            nc.scalar.activation(
                                out=p_t[:],
                                in_=s_ps[:],
                                func=mybir.ActivationFunctionType.Exp,
                                scale=SCALE,
                                bias=t_bias[:],
                            )
                        pb_t[(pi, j)] = p_t

                sp = 0

                def ensure(need_idx):
                    nonlocal sp
                    target = min(need_idx + 1 + LOOK, len(sunits))
                    while sp < target:
                        s_unit(sp)
                        sp += 1

                def drain_half(sl, dv, o_cur, do_l):
                    if do_l:
                        lt = ost.tile([2, QCH], F32, tag="lt", name=f"lt{sl}")
                        nc.vector.tensor_copy(lt[:], l_ps[sl][:])
                        nc.sync.dma_start(l_out[sl], lt[:])
                    for q in range(2):
                        ot = ost.tile(
                            [NP, 512], F32, tag=f"ot{q}", name=f"ot{sl}_{dv}_{q}"
                        )
                        tailq = sl == NSLOT - 1 and q == 1
                        if tailq:
                            nc.scalar.activation(
                                out=ot[:], in_=o_cur[q][:],
                                func=mybir.ActivationFunctionType.Copy,
                            )
                            nc.scalar.dma_start(
                                o_r[:, sl * 2 + q, dv * 512 : (dv + 1) * 512],
                                ot[:],
                            )
                        else:
                            nc.vector.tensor_copy(ot[:], o_cur[q][:])
                            nc.sync.dma_start(
                                o_r[:, sl * 2 + q, dv * 512 : (dv + 1) * 512],
                                ot[:],
                            )

                # Every slot accumulates O in two dv-half passes over its j
                # blocks: halves O's PSUM footprint (2 banks, enabling the
                # 5-deep score-bank runahead) and overlaps each dv0 drain
                # with the dv1 matmuls. Pass dv1 prefetches the NEXT slot's
                # S units between its AV blocks.
                NS = len(sunits)
                for sl in range(NSLOT):
                    pi, inp = sl // 2, sl % 2
                    qo = inp * QCH
                    soff_next = soff[(sl + 1) // 2] if sl + 1 < NSLOT else NS - 1
                    l_ps[sl] = lps.tile([2, QCH], F32, tag="l", name=f"l{sl}")
                    for dv in range(2):
                        o_cur = [
                            ops.tile(
                                [NP, 512], F32, tag=f"o{q}", name=f"o{sl}_{dv}_{q}"
                            )
                            for q in range(2)
                        ]
                        for j in range(sl + 1):
                            if dv == 0:
                                ensure(soff[pi] + j)
                            else:
                                ensure(min(soff_next + j, NS - 1))
                            first, last = (j == 0), (j == sl)
                            if inp == 1 and j == sl:
                                pt, coff = pbo_t[pi], 0
                            else:
                                pt, coff = pb_t[(pi, j)], qo
                            if dv == 0:
                                nc.tensor.matmul(
                                    l_ps[sl][:],
                                    t_ones[:],
                                    pt[:, coff : coff + QCH],
                                    start=first,
                                    stop=last,
                                )
                            for q in range(2):
                                nc.tensor.matmul(
                                    o_cur[q][:],
                                    pt[:, coff + q * NP : coff + (q + 1) * NP],
                                    v_res[:, j, dv * 512 : (dv + 1) * 512],
                                    start=first,
                                    stop=last,
                                )
                        drain_half(sl, dv, o_cur, do_l=(dv == 0))
    nc.compile()
    return nc


def _get_nc():
    if "nc" not in _NC_CACHE:
        _NC_CACHE["nc"] = _build_nc()
    return _NC_CACHE["nc"]


def kernel(x, W_query, W_key, W_value):
    x = np.asarray(x, dtype=np.float32)
    # fold Wq/Wk: scores = x_q @ (Wq^T Wk) @ x_k^T; device computes
    # C^T[e, k'] with stationary wgt[d, e] = (Wk^T @ Wq)[d, e]
    G = (
        np.asarray(W_key, dtype=np.float64).T @ np.asarray(W_query, dtype=np.float64)
    ).astype(BF_NP)
    wgt_a = np.ascontiguousarray(G.reshape(DP, NP, D).transpose(1, 0, 2))
    wvt_f = np.asarray(W_value, dtype=np.float32).T.astype(BF_NP)  # [D, D]
    wvt_a = np.ascontiguousarray(
        wvt_f.reshape(DP, NP, 4, 256).transpose(2, 1, 0, 3)
    )

    ones_a = np.ones((NP, 2), dtype=BF_NP)
    k_l = np.arange(NP)[:, None]
    q_l = np.arange(QCH)[None, :]

    in_maps = []
    for c in range(8):
        b, h = c // 2, c % 2
        xb = x[b]
        # queries bf16: xt[p, pi, dp, qw] = x[b, pi*512+qw, dp*128+p]
        xt_t = np.ascontiguousarray(
            xb.reshape(NPAIR, QW, DP, NP).transpose(3, 0, 2, 1).astype(BF_NP)
        )
        # keys (parity h): fine slabs for keys 0-511, coarse for 512-1023
        xkv = xb[h::2].astype(BF_NP)  # [KP, D]
        xka_t = np.ascontiguousarray(
            xkv[:512].reshape(4, NP, DP, NP).transpose(0, 3, 2, 1)
        )
        xkb_t = np.ascontiguousarray(
            xkv[512:].reshape(512, DP, NP).transpose(2, 1, 0)
        )
        mask_a = np.ones((NP, QW), dtype=BF_NP)
        mask_a[:, 0:QCH] = (q_l >= 2 * k_l + h).astype(BF_NP)
        in_maps.append(
            {
                "xt": xt_t,
                "xka": xka_t,
                "xkb": xkb_t,
                "wgt": wgt_a,
                "wvt": wvt_a,
                "mask": mask_a,
                "ones": ones_a,
            }
        )

    nc = _get_nc()
    res = run_bass_kernel_spmd(nc, in_maps, core_ids=list(range(8)))
    _NC_CACHE["last_res"] = res
    if res.exec_time_ns is not None:
        print(f"HW exec time: {res.exec_time_ns} ns")

    out = np.empty((B, S, D), dtype=np.float32)
    for b in range(B):
        o0 = res.results[2 * b]["o"]
        o1 = res.results[2 * b + 1]["o"]
        l0 = res.results[2 * b]["l"][:, 0, :].reshape(S, 1)
        l1 = res.results[2 * b + 1]["l"][:, 0, :].reshape(S, 1)
        out[b] = (o0 + o1) / (l0 + l1)
    return out
